# revision 41
# baseline (speedup 1.0000x reference)
"""StyleGAN2-style modulated conv (per-sample 3x3, 256->256 ch, 128x128) on 8 TRN2 cores.

Data-parallel over batch: core c computes sample c entirely on-chip.
1D Winograd F(2,3) along H cuts PE work 1.5x vs direct shift-and-matmul:
per output row-pair only 4 H-taps x 3 W-taps of N=512 matmuls (vs 2x9).

v2.5:
- PE warm-up dummy matmuls during the preamble/DMA wait (HAM stays 8/8).
- style smalls then per-(ot,it) weight chunks then x stages on the sync
  HWDGE queue; PE transposes start per-chunk with raw fp16 evictions.
- style modulation rides the x cast: the ACT fp32->fp16 cast scales by
  st[i] (per-partition, free), so weights stay raw and nothing on the
  weight/transpose path waits for style.
- winograd weight build: 5 fp16 DVE ops per i-half (taps 0/3 alias wm).
- eviction (baseline split): ACT copies P1/P2, DVE PSUM-side TTs, GpSimd
  the y0 add, inline ACT demod+store.  Group 0 defers demod (dn off the
  startup critical path); the last group fuses dn into 4 scaled ACT
  copies for the shortest tail.
- wsq squares on ACT, adds on GpSimd; denominator matmul contracts
  against style^2.
"""

import numpy as np
from contextlib import ExitStack

import concourse.bass as bass
import concourse.mybir as mybir
import concourse.tile as tile
from concourse import bacc
from concourse.masks import make_identity

FP32 = mybir.dt.float32
FP16 = mybir.dt.float16
AX = mybir.AxisListType
AF = mybir.ActivationFunctionType
OP = mybir.AluOpType

B = 8
CI = 256
CO = 256
H = 128
W = 128
KS = 3
Z = 512
NKK = KS * KS          # 9 kernel taps
IT = CI // 128         # 2 input-channel tiles
OT = CO // 128         # 2 output-channel tiles
RG = 16                # output rows per group
G = H // RG            # 8 row groups
NP = RG // 2           # 8 winograd row-pairs per group
WP = W + 2             # padded width (zero cols at 0 and WP-1)
TH = 4                 # winograd taps along H
KH = KS * CO           # 768: one kh-slab (3 kw x 256 o)
EPS = 1e-8


def build_nc() -> bass.Bass:
    nc = bacc.Bacc("TRN2", target_bir_lowering=False, debug=False)
    x_d = nc.dram_tensor("x", [CI, H, W], FP32, kind="ExternalInput")
    w_d = nc.dram_tensor("w", [Z], FP32, kind="ExternalInput")
    wt_d = nc.dram_tensor("weight", [CO, CI, KS, KS], FP32, kind="ExternalInput")
    aw_d = nc.dram_tensor("affine_w", [CI, Z], FP32, kind="ExternalInput")
    ab_d = nc.dram_tensor("affine_b", [CI], FP32, kind="ExternalInput")
    y_d = nc.dram_tensor("y", [CO, H, W], FP32, kind="ExternalOutput")

    with tile.TileContext(nc) as tc, ExitStack() as ctx:
        singles = ctx.enter_context(tc.tile_pool(name="singles", bufs=1))
        work = ctx.enter_context(tc.tile_pool(name="work", bufs=1))
        xstage = ctx.enter_context(tc.tile_pool(name="xstage", bufs=3))
        xpool = ctx.enter_context(tc.tile_pool(name="xg", bufs=4))
        xtpool = ctx.enter_context(tc.tile_pool(name="xt", bufs=6))

        zrow = singles.tile([128, WP], FP16)
        nc.vector.memset(zrow, 0.0)
        ident = singles.tile([128, 128], FP32)
        make_identity(nc, ident)
        eps_t = singles.tile([128, 1], FP32)
        nc.vector.memset(eps_t, EPS)

        # ---- PE warm-up: dummy matmuls during the input-DMA wait keep
        # the HAM clock-gate at 8/8 so transposes + early conv run at
        # 2.4GHz (results unused) ----
        with tc.tile_pool(name="warm", bufs=1, space="PSUM") as wp:
            wt_warm = wp.tile([128, 128], FP32, name="warm", tag="warm")
            for _ in range(24):
                nc.tensor.matmul(wt_warm, lhsT=ident, rhs=ident,
                                 start=True, stop=True)

        # ---- style smalls first on the sync queue (style gates the
        # x-cast scale), then weight chunks ----
        wb = singles.tile([128, Z], FP32)
        w_ap = w_d[:]
        nc.sync.dma_start(
            out=wb,
            in_=bass.AP(tensor=w_ap.tensor, offset=w_ap.offset, ap=[[0, 128], [1, Z]]),
        )
        af, ab1 = [], []
        for it in range(IT):
            a = singles.tile([128, Z], FP32, tag=f"af{it}")
            nc.sync.dma_start(out=a, in_=aw_d[it * 128:(it + 1) * 128, :])
            af.append(a)
            abt = singles.tile([128, 1], FP32, tag=f"ab{it}")
            nc.sync.dma_start(
                out=abt, in_=ab_d[it * 128:(it + 1) * 128].rearrange("(p o) -> p o", o=1)
            )
            ab1.append(abt)

        # ---- weight DMA (sync queue), one chunk per (ot, it) ----
        wo_ctx = tc.tile_pool(name="wo", bufs=1)
        wopool = wo_ctx.__enter__()
        HALF = (CI // IT) * NKK  # 1152 elements per i-half
        wo = [[None] * IT for _ in range(OT)]

        def load_wo_half(it):
            for ot in range(OT):
                t = wopool.tile([128, HALF], FP32, name=f"wo{ot}{it}",
                                tag=f"wo{ot}{it}")
                wo[ot][it] = t
                nc.sync.dma_start(
                    out=t,
                    in_=wt_d[
                        ot * 128:(ot + 1) * 128, it * 128:(it + 1) * 128
                    ].rearrange("o i kh kw -> o (i kh kw)"),
                )

        load_wo_half(0)

        # ---- x row-group loads: fp32 stage (sync); the ACT cast to
        # padded fp16 applies the per-channel style scale for free
        # (y = sum_i w[i,o] * (st[i] x[i]) == modulated conv) ----
        xg_tiles: dict = {}
        xt_tiles: dict = {}

        def lg_dma(g: int):
            r0 = g * RG
            lo, hi = r0 - 1, r0 + RG + 1
            clo, chi = max(lo, 0), min(hi, H)
            nrows = chi - clo
            gs = []
            for it in range(IT):
                stg = xstage.tile([128, RG + 2, W], FP32, name="stg", tag="stg")
                nc.sync.dma_start(
                    out=stg[:, 0:nrows, :],
                    in_=x_d[it * 128:(it + 1) * 128, clo:chi, :],
                )
                gs.append(stg)
            xg_tiles[g] = (gs, lo, hi, clo, chi)

        def lg_cast(g: int):
            gs, lo, hi, clo, chi = xg_tiles[g]
            nrows = chi - clo
            gx = []
            for it in range(IT):
                t = xpool.tile([128, RG + 2, WP], FP16, name="xg", tag="xg")
                nc.scalar.mul(
                    out=t[:, clo - lo: chi - lo, 1:W + 1],
                    in_=gs[it][:, 0:nrows, :], mul=st[it],
                )
                nc.gpsimd.tensor_copy(out=t[:, :, 0], in_=zrow[:, 0:RG + 2])
                nc.gpsimd.tensor_copy(out=t[:, :, WP - 1], in_=zrow[:, 0:RG + 2])
                if lo < 0:
                    nc.gpsimd.tensor_copy(out=t[:, 0, :], in_=zrow)
                if hi > H:
                    nc.gpsimd.tensor_copy(out=t[:, RG + 1, :], in_=zrow)
                gx.append(t)
            xg_tiles[g] = gx
            xt_tiles[g] = []

        lg_dma(0)
        load_wo_half(1)
        lg_dma(1)
        lg_dma(2)

        # ---- style: st = w @ affine_w.T + affine_b + 1 (per i-half) ----
        st, st2 = [], []

        def style(it):
            stt = work.tile([128, Z], FP32, name="stt", tag="styletmp")
            nc.vector.tensor_mul(stt, af[it], wb)
            s = singles.tile([128, 1], FP32, name="s", tag=f"st{it}")
            nc.vector.reduce_sum(s, stt, axis=AX.X)
            nc.vector.tensor_add(s, s, ab1[it])
            nc.vector.tensor_scalar_add(s, s, 1.0)
            st.append(s)
            s2 = singles.tile([128, 1], FP32, name="s2", tag=f"st2{it}")
            nc.vector.tensor_mul(s2, s, s)
            st2.append(s2)

        style(0)
        style(1)

        # ---- PE transpose + raw fp16 eviction (style rides the x cast) ----
        wm = [
            singles.tile([128, NKK * CO], FP16, name=f"wm{it}", tag=f"wm{it}")
            for it in range(IT)
        ]
        ws = [
            singles.tile([128, 2 * KH], FP16, name=f"ws{it}", tag=f"ws{it}")
            for it in range(IT)
        ]

        def transpose_half(it):
            with tc.tile_pool(name=f"tpsum{it}", bufs=7, space="PSUM") as tps:
                for ot in range(OT):
                    for kk in range(NKK):
                        pt = tps.tile([128, 128], FP32, name="pt", tag="pt")
                        src = wo[ot][it].rearrange("o (i k) -> o i k", k=NKK)[
                            :, :, kk
                        ]
                        nc.tensor.transpose(out=pt, in_=src, identity=ident)
                        dst = wm[it][:, kk * CO + ot * 128: kk * CO + (ot + 1) * 128]
                        if it == 0:
                            nc.vector.tensor_copy(dst, pt)
                        else:
                            nc.scalar.copy(out=dst, in_=pt)

        def wbuild(it):
            # taps: t0 = k0, t1 = 0.5(k0+k1+k2), t2 = 0.5(k1-k0-k2),
            # t3 = k2 (raw weights; style lives in the x tiles)
            k0 = wm[it][:, 0 * KH:1 * KH]
            k1 = wm[it][:, 1 * KH:2 * KH]
            k2 = wm[it][:, 2 * KH:3 * KH]
            u1 = work.tile([128, KH], FP16, name="u1", tag=f"u1{it}")
            aa = work.tile([128, KH], FP16, name="aa", tag=f"aa{it}")
            nc.vector.tensor_add(u1, k0, k2)
            nc.vector.tensor_add(aa, u1, k1)
            nc.vector.tensor_scalar_mul(ws[it][:, 0:KH], aa, 0.5)
            nc.vector.tensor_sub(aa, k1, u1)
            nc.vector.tensor_scalar_mul(ws[it][:, KH:2 * KH], aa, 0.5)

        def lhsT(it, t, kw, ot):
            base = kw * CO + ot * 128
            if t == 0:
                return wm[it][:, base:base + 128]
            if t == 3:
                return wm[it][:, 2 * KH + base:2 * KH + base + 128]
            return ws[it][:, (t - 1) * KH + base:(t - 1) * KH + base + 128]

        transpose_half(0)
        wbuild(0)
        lg_cast(0)
        transpose_half(1)

        # ---- wsq[it][i, o] = sum_kk w^2: squares on ACT, adds on GpSimd
        # (style^2 enters via the denominator matmul rhs) ----
        wsq = []
        for it in range(IT):
            q = singles.tile([128, CO], FP32, name="wsq", tag=f"wsq{it}")
            nc.scalar.square(q, wm[it][:, 0:CO])
            for kk in range(1, NKK):
                slab = wm[it][:, kk * CO:(kk + 1) * CO]
                tmp = work.tile([128, CO], FP32, name="wsqt",
                                tag=f"wsqt{it}{kk % 2}")
                nc.scalar.square(tmp, slab)
                nc.gpsimd.tensor_add(q, q, tmp)
            wsq.append(q)

        lg_cast(1)

        # ---- x transform: F(2,3) rows, 4 plain fp16 TTs on DVE ----
        def transform_it(g: int, it: int):
            xg = xg_tiles[g][it]
            xt = xtpool.tile([128, TH, NP, WP], FP16, name="xt", tag="xt")
            xt_tiles[g].append(xt)
            ev = xg.rearrange("p (r c) w -> p r c w", c=2)  # [128, 9, 2, WP]
            d0 = ev[:, 0:NP, 0, :]   # rows 0,2,..,14
            d1 = ev[:, 0:NP, 1, :]   # rows 1,3,..,15
            d2 = ev[:, 1:NP + 1, 0, :]  # rows 2,4,..,16
            d3 = ev[:, 1:NP + 1, 1, :]  # rows 3,5,..,17
            nc.vector.tensor_sub(xt[:, 0], d0, d2)
            nc.vector.tensor_add(xt[:, 1], d1, d2)
            nc.vector.tensor_sub(xt[:, 2], d1, d2)
            nc.vector.tensor_sub(xt[:, 3], d1, d3)

        transform_it(0, 0)
        transform_it(0, 1)
        wbuild(1)

        # ---- conv machinery ----
        wo_ctx.__exit__(None, None, None)
        opool = ctx.enter_context(tc.tile_pool(name="og", bufs=5))
        spool = ctx.enter_context(tc.tile_pool(name="escr", bufs=2))
        cpsum = ctx.enter_context(tc.tile_pool(name="cpsum", bufs=7, space="PSUM"))
        dpsum = ctx.enter_context(tc.tile_pool(name="dpsum", bufs=1, space="PSUM"))
        dn = []

        def half_matmuls(g: int, ot: int, h: int, it_outer=False):
            # it_outer: all it0 taps first — group 0 startup can issue
            # its it0 matmuls before the it1 weight path has landed.
            P = [
                cpsum.tile([128, 512], FP32, name="pg", tag="pg")
                for _ in range(TH)
            ]
            order = (
                [(it, t) for it in range(IT) for t in range(TH)]
                if it_outer else
                [(it, t) for t in range(TH) for it in range(IT)]
            )
            for it, t in order:
                xt = xt_tiles[g][it]
                for kw in range(KS):
                    nc.tensor.matmul(
                        P[t],
                        lhsT=lhsT(it, t, kw, ot),
                        rhs=xt[:, t, 4 * h:4 * h + 4, kw:kw + W],
                        start=(it == 0 and kw == 0),
                        stop=(it == IT - 1 and kw == KS - 1),
                    )
            return P

        def store_og(og, g, ot, h):
            yv = y_d[ot * 128:(ot + 1) * 128].rearrange(
                "o (r j) w -> o r j w", j=2
            )
            r0 = g * (RG // 2) + h * TH
            for j in range(2):
                nc.sync.dma_start(
                    out=yv[:, r0:r0 + TH, j, :], in_=og[:, j]
                )

        def evict(g: int, ot: int, h: int, mode="demod", it_outer=False):
            # y0 = (P0+P1+P2)*dn, y1 = (P1-P2-P3)*dn.
            # "demod": baseline split — ACT copies P1/P2 raw, DVE combines
            #   (PSUM-side TTs), GpSimd the y0 add, inline ACT demod+store.
            # "raw": same but no demod/store (group 0 defers until dn).
            # "fused": dn folded into 4 scaled ACT copies + 4 plain DVE
            #   TTs — shortest post-matmul chain, used for the last group.
            P = half_matmuls(g, ot, h, it_outer)
            Pv = [p.rearrange("p (a b) -> p a b", b=W) for p in P]
            og = opool.tile([128, 2, TH, W], FP32, name="og", tag="og")
            c1 = spool.tile([128, TH, W], FP32, name="c1", tag="c1")
            c2 = spool.tile([128, TH, W], FP32, name="c2", tag="c2")
            u = spool.tile([128, TH, W], FP32, name="u", tag="u")
            v = spool.tile([128, TH, W], FP32, name="v", tag="v")
            if mode == "fused":
                c0 = spool.tile([128, TH, W], FP32, name="c0", tag="c0")
                c3 = spool.tile([128, TH, W], FP32, name="c3", tag="c3")
                nc.scalar.mul(out=c0, in_=Pv[0], mul=dn[ot])
                nc.scalar.mul(out=c1, in_=Pv[1], mul=dn[ot])
                nc.scalar.mul(out=c2, in_=Pv[2], mul=dn[ot])
                nc.scalar.mul(out=c3, in_=Pv[3], mul=dn[ot])
                nc.vector.tensor_add(u, c0, c1)
                nc.vector.tensor_add(og[:, 0], u, c2)
                nc.vector.tensor_sub(v, c1, c2)
                nc.vector.tensor_sub(og[:, 1], v, c3)
                store_og(og, g, ot, h)
                return og
            nc.scalar.copy(out=c1, in_=Pv[1])
            nc.vector.tensor_add(u, c1, Pv[0])
            nc.scalar.copy(out=c2, in_=Pv[2])
            nc.gpsimd.tensor_add(og[:, 0], u, c2)
            nc.vector.tensor_sub(v, c1, c2)
            nc.vector.tensor_sub(og[:, 1], v, Pv[3])
            if mode == "demod":
                nc.scalar.mul(out=og, in_=og, mul=dn[ot])
                store_og(og, g, ot, h)
            return og

        # ---- group 0: raw evictions (dn off the critical path),
        # transforms for g1 interleaved; denominators after g0's MMs ----
        ogs = []
        ogs.append((evict(0, 0, 0, mode="raw", it_outer=True), 0, 0))
        ogs.append((evict(0, 0, 1, mode="raw", it_outer=True), 0, 1))
        transform_it(1, 0)
        ogs.append((evict(0, 1, 0, mode="raw", it_outer=True), 1, 0))
        transform_it(1, 1)
        ogs.append((evict(0, 1, 1, mode="raw", it_outer=True), 1, 1))

        def emit_denom():
            for ot in range(OT):
                pd = dpsum.tile([128, 1], FP32, name="pd", tag="pd")
                for it in range(IT):
                    nc.tensor.matmul(
                        pd,
                        lhsT=wsq[it][:, ot * 128:(ot + 1) * 128],
                        rhs=st2[it],
                        start=(it == 0),
                        stop=(it == IT - 1),
                    )
                dcol = singles.tile([128, 1], FP32, name="dn", tag=f"dn{ot}")
                nc.scalar.activation(out=dcol, in_=pd, func=AF.Sqrt, bias=eps_t)
                nc.vector.reciprocal(dcol, dcol)
                dn.append(dcol)

        emit_denom()
        for og, ot, h in ogs:
            nc.scalar.mul(out=og, in_=og, mul=dn[ot])
            store_og(og, 0, ot, h)

        for g in range(1, G):
            if g + 2 < G:
                lg_dma(g + 2)
            if g + 1 < G:
                lg_cast(g + 1)
            mode = "fused" if g == G - 1 else "demod"
            evict(g, 0, 0, mode)
            evict(g, 0, 1, mode)
            if g + 1 < G:
                transform_it(g + 1, 0)
            evict(g, 1, 0, mode)
            if g + 1 < G:
                transform_it(g + 1, 1)
            evict(g, 1, 1, mode)
    nc.finalize()
    return nc


_CACHE: dict = {}


def _get_nc() -> bass.Bass:
    if "nc" not in _CACHE:
        _CACHE["nc"] = build_nc()
    return _CACHE["nc"]


def make_in_maps(x, w, weight, affine_w, affine_b):
    x = np.ascontiguousarray(x, dtype=np.float32)
    w = np.ascontiguousarray(w, dtype=np.float32)
    weight = np.ascontiguousarray(weight, dtype=np.float32)
    affine_w = np.ascontiguousarray(affine_w, dtype=np.float32)
    affine_b = np.ascontiguousarray(affine_b, dtype=np.float32)
    return [
        {
            "x": x[c],
            "w": w[c],
            "weight": weight,
            "affine_w": affine_w,
            "affine_b": affine_b,
        }
        for c in range(B)
    ]


def run_on_hw(inputs: dict, trace: bool = False, tmpdir: str | None = None):
    from concourse.bass_utils import run_bass_kernel_spmd

    nc = _get_nc()
    in_maps = make_in_maps(**inputs)
    res = run_bass_kernel_spmd(
        nc, in_maps, core_ids=list(range(B)), trace=trace, tmpdir=tmpdir
    )
    y = np.stack([r["y"] for r in res.results], axis=0)
    return y, res


def kernel(x, w, weight, affine_w, affine_b):
    y, _ = run_on_hw(
        dict(x=x, w=w, weight=weight, affine_w=affine_w, affine_b=affine_b)
    )
    return y
